# revision 45
# baseline (speedup 1.0000x reference)
"""CBAM kernel for Trainium2, 8-way batch-parallel SPMD, f16 data path.

Computes out = x^2 * (att_c[b,c] + sigmoid(conv(spatial_stats))[b,l]) where
att_c = sigmoid(mlp(mean_L x) + mlp(max_L x)), matching the CBAM reference.

Key decisions (all measured on HW, see transcript):
- x staged host-side as float16 in the SBUF tile layout [NB, 128, NT*C]
  (partition = l % 128, col = 256*(l//128) + c) so each batch loads/stores
  via two dma_starts of 128 x 8KB contiguous rows. Halves HBM traffic vs
  f32 and kills the baseline's per-DMA sequencing bottleneck.
- Spatial sum/max over C and chan-max over L run as TT fold trees (DVE
  TensorTensor hits the 2x f16 mode; TensorReduce-3D and
  scalar_tensor_tensor only run at 1x; first folds are split per DMA half
  so they start as each half lands).
- The final combine is satt = att + sig (per-tile: DVE tensor_scalar at
  4x for SATT_DVE tiles, ACT identity-with-bias for the rest) followed by
  one bulk TT multiply ot = satt * x^2 (DVE + a Pool tail; Pool's TT is
  ~4x slower per element but runs off the critical engine).
- Emission is software-pipelined: stats(b) -> final(b-1) -> sq(b) so the
  ACT stats chain never queues behind bulk work and the cross-engine satt
  join never stalls DVE head-of-line.

Engine load per batch (measured): DVE ~23us (trees + satt share + bulk
mul), ACT ~20us (x^2 + satt share + sigmoids), PE ~8us (chan-sum matmuls,
transposes, MLP, Toeplitz conv), Pool ~9us (mul tail + store DMA).
"""

import numpy as np
from contextlib import ExitStack

import concourse.bacc as bacc
import concourse.bass as bass
import concourse.tile as tile
import concourse.mybir as mybir
from concourse.bass_utils import run_bass_kernel_spmd

AF = mybir.ActivationFunctionType
ALU = mybir.AluOpType
AX = mybir.AxisListType
FP32 = mybir.dt.float32
FP16 = mybir.dt.float16

N_CORES = 8
B_FULL = 32
NB = B_FULL // N_CORES  # batches per core = 4
L = 4096
C = 256
HID = 16
P = 128
NT = L // P  # 32 L-tiles per batch
F = NT * C   # 8192 free columns per batch

_CACHE: dict = {}


SATT_DVE = 8      # tiles whose att+sig runs on DVE tensor_scalar (rest ACT)
MUL_DVE = 7168    # columns of the final multiply on DVE (rest Pool)
SQ_ACT = 8192     # columns of x^2 on ACT (rest Pool)


def _build_body(ctx: ExitStack, tc, out_d, x_d, w1_d, b1_d, w2b_d, cm_d, cc_d,
                ones_d, id_d, rc_d, reps=1):
    nc = tc.nc

    const = ctx.enter_context(tc.tile_pool(name="const", bufs=1))
    xpool = ctx.enter_context(tc.tile_pool(name="x", bufs=2))
    sqpool = ctx.enter_context(tc.tile_pool(name="sq", bufs=2))
    stpool = ctx.enter_context(tc.tile_pool(name="satt", bufs=2))
    opool = ctx.enter_context(tc.tile_pool(name="outt", bufs=2))
    mpool = ctx.enter_context(tc.tile_pool(name="maxtree", bufs=2))
    spool = ctx.enter_context(tc.tile_pool(name="stats", bufs=2))
    apool = ctx.enter_context(tc.tile_pool(name="att", bufs=2))
    pacc = ctx.enter_context(tc.tile_pool(name="pacc", bufs=2, space="PSUM"))
    ptrp = ctx.enter_context(tc.tile_pool(name="ptrp", bufs=2, space="PSUM"))
    pwork = ctx.enter_context(tc.tile_pool(name="pwork", bufs=4, space="PSUM"))

    # param loads ride the scalar queue so the first x load isn't delayed
    w1 = const.tile([P, 2 * (HID + 1)], FP32)
    nc.scalar.dma_start(w1[:], w1_d[:])
    b1 = const.tile([HID + 1, 1], FP32)
    nc.scalar.dma_start(b1[:], b1_d[:])
    w2b = const.tile([HID + 1, C], FP32)
    nc.scalar.dma_start(w2b[:], w2b_d[:])
    cmain = const.tile([P, 2 * P], FP16)
    nc.scalar.dma_start(cmain[:], cm_d[:])
    ccorn = const.tile([P, 4 * P], FP16)
    nc.scalar.dma_start(ccorn[:], cc_d[:])
    ones = const.tile([P, P], FP32)
    nc.scalar.dma_start(ones[:], ones_d[:])
    ident = const.tile([P, P], FP16)
    nc.scalar.dma_start(ident[:], id_d[:])
    redcol = const.tile([P, 1], FP16)
    nc.scalar.dma_start(redcol[:], rc_d[:])

    def spatial_fold1(xt, fa, op, half):
        """First c-fold (256 -> 128) for one DMA half of x (16 tiles)."""
        lo = F // 2 * half
        x4 = xt[:, lo:lo + F // 2].rearrange("p (t h c) -> p t h c",
                                             h=2, c=128)
        nc.vector.tensor_tensor(
            fa[:, 2048 * half:2048 * (half + 1)].rearrange(
                "p (t h c) -> p t h c", h=1, c=128),
            x4[:, :, 0:1, :], x4[:, :, 1:2, :], op=op)

    def spatial_rest(fa, fb, out, op, pool_tail=False):
        """Folds c: 128 -> 8 through fa/fb ping-pong, then a cheap tail.

        pool_tail moves the two small late folds to the idle Pool engine.
        """
        seq = [(fa, 4096), (fb, 2048), (fa, 1024), (fb, 512), (fa, 256)]
        for i in range(1, 5):
            sbuf, sw = seq[i - 1]
            dbuf, dw = seq[i]
            ch = sw // NT // 2
            s4 = sbuf[:, 0:sw].rearrange("p (t h c) -> p t h c", h=2, c=ch)
            eng = nc.gpsimd if (pool_tail and i >= 3) else nc.vector
            eng.tensor_tensor(
                dbuf[:, 0:dw].rearrange("p (t h c) -> p t h c", h=1, c=ch),
                s4[:, :, 0:1, :], s4[:, :, 1:2, :], op=op)
        with nc.allow_low_precision(reason="f16 spatial stats feed sigmoid"):
            nc.vector.tensor_reduce(
                out[:], fa[:, 0:256].rearrange("p (t c) -> p t c", c=8),
                axis=AX.X, op=op)

    def emit_stats(b):
        st = {}
        xt = xpool.tile([P, F], FP16, tag="x", name=f"x{b}")
        # batch 0's second half rides the idle gpsimd queue so both halves'
        # descriptor generation overlaps (fill time); later batches load
        # far enough ahead that the sync queue alone keeps up
        nc.sync.dma_start(xt[:, 0:F // 2], x_d[b, :, 0:F // 2])
        q2 = nc.gpsimd if b == 0 else nc.sync
        q2.dma_start(xt[:, F // 2:F], x_d[b, :, F // 2:F])
        st["xt"] = xt

        # channel sum over L (PE): 16 matmuls of [1, 512]; even tiles land
        # in cols 0:256, odd tiles in 256:512 (folded by DVE below)
        pcs2 = pacc.tile([1, 2 * C], FP32, tag="pcs")
        for j in range(NT // 2):
            nc.tensor.matmul(pcs2[:], redcol[:], xt[:, 2 * C * j:2 * C * (j + 1)],
                             start=(j == 0), stop=(j == NT // 2 - 1),
                             skip_group_check=True)

        # per-half first folds start as soon as each DMA half lands
        mb = mpool.tile([P, F // 2], FP16, tag="mb")
        fa = mpool.tile([P, 4096], FP16, tag="fa")
        fb = mpool.tile([P, 2048], FP16, tag="fb")
        ga = mpool.tile([P, 4096], FP16, tag="ga")
        gb = mpool.tile([P, 2048], FP16, tag="gb")
        for h in range(2):
            lo = F // 2 * h
            # chan-max: fold tiles {t, t+8} within this half
            nc.vector.tensor_max(mb[:, 2048 * h:2048 * (h + 1)],
                                 xt[:, lo:lo + 2048], xt[:, lo + 2048:lo + 4096])
            spatial_fold1(xt, fa, ALU.max, h)
            spatial_fold1(xt, ga, ALU.add, h)

        # chan-max tree: fold mb 4096 -> 256 (contiguous halves)
        w = F // 4
        while w >= C:
            nc.vector.tensor_max(mb[:, 0:w], mb[:, 0:w], mb[:, w:2 * w])
            w //= 2
        # cross-partition max via PE transposes + small DVE reduces
        pmaxT = ptrp.tile([P, 2 * P], FP16, tag="pmaxT")
        for h in range(2):
            nc.tensor.transpose(pmaxT[:, P * h:P * (h + 1)],
                                mb[:, P * h:P * (h + 1)], ident[:])
        stats_cm = spool.tile([P, 4], FP32, tag="stats_cm")
        # one 3D reduce fills both max columns (strided [p, 2, 1] out view)
        nc.vector.tensor_reduce(
            stats_cm[:].rearrange("p (h v) -> p h v", v=2)[:, :, 1:2],
            pmaxT[:].rearrange("p (h c) -> p h c", c=P),
            axis=AX.X, op=ALU.max)

        # spatial max + sum trees (DVE)
        max_s = spool.tile([P, NT], FP16, tag="max_s")
        spatial_rest(fa, fb, max_s, ALU.max)
        # avg fold: [1, 512] psum -> sbuf copy -> [1, 256] f16 (Pool add)
        sr512 = spool.tile([1, 2 * C], FP16, tag="sr512")
        nc.scalar.activation(sr512[:], pcs2[:], AF.Copy)
        avg_row = spool.tile([1, C], FP16, tag="avg_row")
        nc.gpsimd.tensor_add(avg_row[:], sr512[0:1, 0:C], sr512[0:1, C:2 * C])
        sum_s = spool.tile([P, NT], FP16, tag="sum_s")
        spatial_rest(ga, gb, sum_s, ALU.add)

        # avg transposes into channel-major + ACT copies
        pavgT = pwork.tile([P, 4], FP16, tag="pwork")
        for h in range(2):
            nc.tensor.transpose(pavgT[:, 2 * h:2 * h + 1],
                                avg_row[0:1, P * h:P * (h + 1)],
                                ident[0:1, 0:1])
        for h in range(2):
            nc.scalar.activation(stats_cm[:, 2 * h:2 * h + 1],
                                 pavgT[:, 2 * h:2 * h + 1], AF.Copy)

        # shared MLP (f32, tiny): row HID carries the 2*b2 constant
        ph = pwork.tile([HID + 1, 2], FP32, tag="pwork")
        nc.tensor.matmul(ph[:], w1[:, 0:HID + 1], stats_cm[:, 0:2],
                         start=True, stop=False, skip_group_check=True)
        nc.tensor.matmul(ph[:], w1[:, HID + 1:2 * (HID + 1)], stats_cm[:, 2:4],
                         start=False, stop=True, skip_group_check=True)
        hsb = spool.tile([HID + 1, 2], FP32, tag="hsb")
        nc.scalar.activation(hsb[:], ph[:], AF.Relu, bias=b1[:])
        h2 = spool.tile([HID + 1, 1], FP32, tag="h2")
        nc.vector.tensor_add(h2[:], hsb[:, 0:1], hsb[:, 1:2])
        h2r = spool.tile([HID + 1, P], FP32, tag="h2r")
        nc.vector.tensor_scalar_mul(h2r[:], ones[0:HID + 1, :], h2[:])
        po = pwork.tile([P, C], FP32, tag="pwork")
        nc.tensor.matmul(po[:], h2r[:], w2b[:], start=True, stop=True,
                         skip_group_check=True)
        att = apool.tile([P, C], FP16, tag="att")
        nc.scalar.activation(att[:], po[:], AF.Sigmoid)
        st["att"] = att

        # spatial conv over L: banded-Toeplitz matmuls (f16 in)
        pc = pwork.tile([P, NT], FP32, tag="pwork")
        nc.tensor.matmul(pc[:, :], cmain[:, 0:P], sum_s[:],
                         start=True, stop=False, skip_group_check=True)
        nc.tensor.matmul(pc[:, :], cmain[:, P:2 * P], max_s[:],
                         start=False, stop=False, skip_group_check=True)
        nc.tensor.matmul(pc[:, 1:NT], ccorn[:, 0:P], sum_s[:, 0:NT - 1],
                         start=False, stop=False, skip_group_check=True)
        nc.tensor.matmul(pc[:, 1:NT], ccorn[:, P:2 * P], max_s[:, 0:NT - 1],
                         start=False, stop=False, skip_group_check=True)
        nc.tensor.matmul(pc[:, 0:NT - 1], ccorn[0:3, 2 * P:3 * P],
                         sum_s[0:3, 1:NT],
                         start=False, stop=False, skip_group_check=True)
        nc.tensor.matmul(pc[:, 0:NT - 1], ccorn[0:3, 3 * P:4 * P],
                         max_s[0:3, 1:NT],
                         start=False, stop=True, skip_group_check=True)
        sig = apool.tile([P, NT], FP32, tag="sig")
        nc.scalar.activation(sig[:], pc[:], AF.Sigmoid)
        st["sig"] = sig
        st["b"] = b
        return st

    def emit_sq(st):
        # x^2: ACT for most columns (emitted after the previous batch's
        # final so the ACT stats chain isn't queued behind it), Pool tail
        sq = sqpool.tile([P, F], FP16, tag="sq")
        xt = st["xt"]
        nc.scalar.activation(sq[:, 0:SQ_ACT // 2], xt[:, 0:SQ_ACT // 2],
                             AF.Square)
        nc.scalar.activation(sq[:, SQ_ACT // 2:SQ_ACT],
                             xt[:, SQ_ACT // 2:SQ_ACT], AF.Square)
        if SQ_ACT < F:
            nc.gpsimd.tensor_tensor(sq[:, SQ_ACT:F], xt[:, SQ_ACT:F],
                                    xt[:, SQ_ACT:F], op=ALU.mult)
        st["sq"] = sq

    def emit_final(st, last=False):
        att, sig, sq = st["att"], st["sig"], st["sq"]
        # satt[:, 256t + c] = att[c] + sig[p, t], processed per half-batch
        # so each half's multiply and store start as soon as its satts land.
        # Within a half: DVE tensor_scalar for the first tiles, ACT
        # identity-with-bias for the rest; mul = DVE + small Pool tail.
        satt = stpool.tile([P, F], FP16, tag="satt")
        ot = opool.tile([P, F], FP16, tag="ot")
        half_t = NT // 2
        hf = F // 2
        sdve = half_t // 2 if last else SATT_DVE // 2 - 1
        pool_cols = 512
        for hh in range(2):
            t0 = half_t * hh
            for t in range(t0, t0 + sdve):
                nc.vector.tensor_scalar_add(satt[:, C * t:C * (t + 1)],
                                            att[:], sig[:, t:t + 1])
            for t in range(t0 + sdve, t0 + half_t):
                nc.scalar.activation(satt[:, C * t:C * (t + 1)], att[:],
                                     AF.Identity, bias=sig[:, t:t + 1])
            lo = hf * hh
            nc.vector.tensor_mul(ot[:, lo:lo + hf - pool_cols],
                                 satt[:, lo:lo + hf - pool_cols],
                                 sq[:, lo:lo + hf - pool_cols])
            nc.gpsimd.tensor_tensor(ot[:, lo + hf - pool_cols:lo + hf],
                                    satt[:, lo + hf - pool_cols:lo + hf],
                                    sq[:, lo + hf - pool_cols:lo + hf],
                                    op=ALU.mult)
            nc.gpsimd.dma_start(out_d[st["b"], :, lo:lo + hf],
                                ot[:, lo:lo + hf])

    # software-pipelined emission: final(b-1) lands after stats(b) so no
    # engine stalls head-of-line on the cross-engine satt join; sq(b) goes
    # last so the ACT stats chain isn't queued behind it
    prev = None
    for b in [b for _ in range(reps) for b in range(NB)]:
        cur = emit_stats(b)
        if prev is not None:
            emit_final(prev)
        emit_sq(cur)
        prev = cur
    emit_final(prev, last=True)


def _build_nc(reps=1):
    nc = bacc.Bacc("TRN2", target_bir_lowering=False, debug=False,
                   enable_asserts=False, num_devices=N_CORES)
    x_d = nc.dram_tensor("xb", [NB, P, F], FP16, kind="ExternalInput").ap()
    w1_d = nc.dram_tensor("w1sb", [P, 2 * (HID + 1)], FP32, kind="ExternalInput").ap()
    b1_d = nc.dram_tensor("b1col", [HID + 1, 1], FP32, kind="ExternalInput").ap()
    w2b_d = nc.dram_tensor("w2b", [HID + 1, C], FP32, kind="ExternalInput").ap()
    cm_d = nc.dram_tensor("convmain", [P, 2 * P], FP16, kind="ExternalInput").ap()
    cc_d = nc.dram_tensor("convcorner", [P, 4 * P], FP16, kind="ExternalInput").ap()
    ones_d = nc.dram_tensor("ones", [P, P], FP32, kind="ExternalInput").ap()
    id_d = nc.dram_tensor("ident", [P, P], FP16, kind="ExternalInput").ap()
    rc_d = nc.dram_tensor("redcol", [P, 1], FP16, kind="ExternalInput").ap()
    out_d = nc.dram_tensor("out", [NB, P, F], FP16, kind="ExternalOutput").ap()

    with tile.TileContext(nc) as tc:
        with ExitStack() as ctx:
            _build_body(ctx, tc, out_d, x_d, w1_d, b1_d, w2b_d, cm_d, cc_d,
                        ones_d, id_d, rc_d, reps=reps)
    nc.compile()
    return nc


def get_nc(reps=1):
    key = f"nc{reps}"
    if key not in _CACHE:
        _CACHE[key] = _build_nc(reps=reps)
    return _CACHE[key]


def _prep_inputs(W1, b1, W2, b2, conv_w):
    """Host-side parameter preprocessing (shared across cores)."""
    W1 = np.asarray(W1, np.float32)
    W2 = np.asarray(W2, np.float32)
    b1 = np.asarray(b1, np.float32)
    b2 = np.asarray(b2, np.float32)
    conv_w = np.asarray(conv_w, np.float32)

    HB = HID + 1
    w1sb = np.zeros((P, 2 * HB), np.float32)
    for h in range(2):
        w1sb[:, HB * h:HB * h + HID] = W1[P * h:P * (h + 1), :]
    w2b = np.concatenate([W2, b2[None, :]], axis=0).astype(np.float32)
    b1col = np.concatenate([b1, [1.0]]).astype(np.float32).reshape(HB, 1)

    # Banded Toeplitz over two adjacent 128-blocks; avg band folds in the
    # 1/C spatial-mean scale (device computes raw channel sums).
    wa = conv_w[:, 0, 0] / C
    wm = conv_w[:, 1, 0]
    Wb_a = np.zeros((2 * P, 2 * P), np.float32)
    Wb_m = np.zeros((2 * P, 2 * P), np.float32)
    for i in range(2 * P):
        for k in range(7):
            j = i + k - 3
            if 0 <= j < 2 * P:
                Wb_a[i, j] = wa[k]
                Wb_m[i, j] = wm[k]
    cmain = np.concatenate([Wb_a[0:P, 0:P].T, Wb_m[0:P, 0:P].T], axis=1)
    # Corner lhsTs in one [128, 512] tensor. The prev-block ("lo") bands use
    # full K=128 (only rows 125-127 nonzero) so the rhs stays at base
    # partition 0 (PE requires base partition in {0, 32, 64}); the
    # next-block ("hi") bands are K=3 at rows 0-2.
    corn = np.zeros((P, 4 * P), np.float32)
    corn[:, 0:P] = Wb_a[P:2 * P, 0:P].T            # prev-block avg
    corn[:, P:2 * P] = Wb_m[P:2 * P, 0:P].T        # prev-block max
    corn[0:3, 2 * P:3 * P] = Wb_a[0:P, P:2 * P].T[0:3, :]   # next-block avg
    corn[0:3, 3 * P:4 * P] = Wb_m[0:P, P:2 * P].T[0:3, :]   # next-block max
    return {
        "w1sb": w1sb,
        "b1col": np.ascontiguousarray(b1col),
        "w2b": w2b,
        "convmain": np.ascontiguousarray(cmain).astype(np.float16),
        "convcorner": np.ascontiguousarray(corn).astype(np.float16),
        "ones": np.ones((P, P), np.float32),
        "ident": np.eye(P, dtype=np.float16),
        "redcol": np.full((P, 1), 1.0 / L, np.float16),
    }


def kernel(x, W1, b1, W2, b2, conv_w):
    nc = get_nc()
    x = np.asarray(x, np.float32)
    params = _prep_inputs(W1, b1, W2, b2, conv_w)
    # Stage x as f16 in the SBUF tile layout: [NB, 128, NT*C] with
    # col = 256 * (l // 128) + c, partition = l % 128.
    xt = x.reshape(B_FULL, NT, P, C).transpose(0, 2, 1, 3).reshape(
        B_FULL, P, F).astype(np.float16)
    in_maps = []
    for c in range(N_CORES):
        m = dict(params)
        m["xb"] = np.ascontiguousarray(xt[NB * c:NB * (c + 1)])
        in_maps.append(m)
    _CACHE["last_in_maps"] = in_maps
    res = run_bass_kernel_spmd(nc, in_maps, list(range(N_CORES)))
    _CACHE["last_results"] = res
    out = np.concatenate([res.results[c]["out"] for c in range(N_CORES)],
                         axis=0)
    # [B, 128, NT*C] f16 -> [B, L, C] f32
    return out.reshape(B_FULL, P, NT, C).transpose(0, 2, 1, 3).reshape(
        B_FULL, L, C).astype(np.float32)


def _pjrt_exec(nc, in_maps, n_warm=2, n_time=8):
    """Build a sharded jit for nc, run it, return (best_wall_s, result)."""
    import time
    import jax
    import concourse.mybir as mybir_
    from concourse.bass2jax import (_bass_exec_p, install_neuronx_cc_hook,
                                    partition_id_tensor)
    from jax.experimental.shard_map import shard_map
    from jax.sharding import Mesh, PartitionSpec

    install_neuronx_cc_hook()
    partition_name = (nc.partition_id_tensor.name
                      if nc.partition_id_tensor else None)
    in_names, out_names, out_avals = [], [], []
    for alloc in nc.m.functions[0].allocations:
        if not isinstance(alloc, mybir_.MemoryLocationSet):
            continue
        name = alloc.memorylocations[0].name
        if alloc.kind == "ExternalInput":
            if name != partition_name:
                in_names.append(name)
        elif alloc.kind == "ExternalOutput":
            out_names.append(name)
            out_avals.append(jax.core.ShapedArray(
                tuple(alloc.tensor_shape), mybir_.dt.np(alloc.dtype)))
    n_params = len(in_names)
    all_in_names = list(in_names) + list(out_names)
    if partition_name is not None:
        all_in_names.append(partition_name)

    def _body(*args):
        operands = list(args)
        if partition_name is not None:
            operands.append(partition_id_tensor())
        return tuple(_bass_exec_p.bind(
            *operands,
            out_avals=tuple(out_avals),
            in_names=tuple(all_in_names),
            out_names=tuple(out_names),
            lowering_input_output_aliases=(),
            sim_require_finite=True,
            sim_require_nnan=True,
            nc=nc,
        ))

    devices = jax.devices()[:N_CORES]
    mesh = Mesh(np.asarray(devices), ("core",))
    nin = n_params + len(out_names)
    sharding = jax.sharding.NamedSharding(mesh, PartitionSpec("core"))
    fn = jax.jit(shard_map(
        _body, mesh=mesh,
        in_specs=(PartitionSpec("core"),) * nin,
        out_specs=(PartitionSpec("core"),) * len(out_names),
        check_rep=False))
    dev_args = [
        jax.device_put(np.concatenate(
            [np.asarray(in_maps[c][nm]) for c in range(N_CORES)], axis=0),
            sharding)
        for nm in in_names
    ]
    for av in out_avals:
        z = np.zeros((N_CORES * av.shape[0], *av.shape[1:]), av.dtype)
        dev_args.append(jax.device_put(z, sharding))

    for _ in range(n_warm):
        out = fn(*dev_args)
        jax.block_until_ready(out)
    best = float("inf")
    for _ in range(n_time):
        t0 = time.perf_counter()
        out = fn(*dev_args)
        jax.block_until_ready(out)
        best = min(best, time.perf_counter() - t0)
    result = np.asarray(out[0])
    return best, result


def bench_repeat(reps=8, n_time=10, in_maps=None):
    """Isolate device exec time: time a module doing the work `reps` times
    in-kernel vs once; slope = steady-state HW time per execution."""
    if in_maps is None:
        in_maps = _CACHE["last_in_maps"]
    t1, _ = _pjrt_exec(get_nc(1), in_maps, n_time=n_time)
    tr, result = _pjrt_exec(get_nc(reps), in_maps, n_time=n_time)
    per_exec_ns = (tr - t1) / (reps - 1) * 1e9
    return per_exec_ns, result, t1 * 1e9, tr * 1e9


# revision 47
# speedup vs baseline: 1.0774x; 1.0774x over previous
"""CBAM kernel for Trainium2, 8-way batch-parallel SPMD, f16 data path.

Computes out = x^2 * (att_c[b,c] + sigmoid(conv(spatial_stats))[b,l]) where
att_c = sigmoid(mlp(mean_L x) + mlp(max_L x)), matching the CBAM reference.

Key decisions (all measured on HW, see transcript):
- x staged host-side as float16 in the SBUF tile layout [NB, 128, NT*C]
  (partition = l % 128, col = 256*(l//128) + c) so each batch loads/stores
  via two dma_starts of 128 x 8KB contiguous rows. Halves HBM traffic vs
  f32 and kills the baseline's per-DMA sequencing bottleneck.
- Spatial sum/max over C and chan-max over L run as TT fold trees (DVE
  TensorTensor hits the 2x f16 mode; TensorReduce-3D and
  scalar_tensor_tensor only run at 1x; first folds are split per DMA half
  so they start as each half lands).
- The final combine is satt = att + sig (per-tile: DVE tensor_scalar at
  4x for SATT_DVE tiles, ACT identity-with-bias for the rest) followed by
  one bulk TT multiply ot = satt * x^2 (DVE + a Pool tail; Pool's TT is
  ~4x slower per element but runs off the critical engine).
- Emission is software-pipelined: stats(b) -> final(b-1) -> sq(b) so the
  ACT stats chain never queues behind bulk work and the cross-engine satt
  join never stalls DVE head-of-line.

Engine load per batch (measured): DVE ~23us (trees + satt share + bulk
mul), ACT ~20us (x^2 + satt share + sigmoids), PE ~8us (chan-sum matmuls,
transposes, MLP, Toeplitz conv), Pool ~9us (mul tail + store DMA).
"""

import numpy as np
from contextlib import ExitStack

import concourse.bacc as bacc
import concourse.bass as bass
import concourse.tile as tile
import concourse.mybir as mybir
from concourse.bass_utils import run_bass_kernel_spmd

AF = mybir.ActivationFunctionType
ALU = mybir.AluOpType
AX = mybir.AxisListType
FP32 = mybir.dt.float32
FP16 = mybir.dt.float16

N_CORES = 8
B_FULL = 32
NB = B_FULL // N_CORES  # batches per core = 4
L = 4096
C = 256
HID = 16
P = 128
NT = L // P  # 32 L-tiles per batch
F = NT * C   # 8192 free columns per batch

_CACHE: dict = {}


SATT_DVE = 8      # tiles whose att+sig runs on DVE tensor_scalar (rest ACT)
MUL_DVE = 7168    # columns of the final multiply on DVE (rest Pool)
SQ_ACT = 8192     # columns of x^2 on ACT (rest Pool)


def _build_body(ctx: ExitStack, tc, out_d, x_d, w1_d, b1_d, w2b_d, cm_d, cc_d,
                ones_d, id_d, rc_d, reps=1):
    nc = tc.nc

    const = ctx.enter_context(tc.tile_pool(name="const", bufs=1))
    xpool = ctx.enter_context(tc.tile_pool(name="x", bufs=2))
    sqpool = ctx.enter_context(tc.tile_pool(name="sq", bufs=2))
    stpool = ctx.enter_context(tc.tile_pool(name="satt", bufs=2))
    opool = ctx.enter_context(tc.tile_pool(name="outt", bufs=2))
    mpool = ctx.enter_context(tc.tile_pool(name="maxtree", bufs=2))
    spool = ctx.enter_context(tc.tile_pool(name="stats", bufs=2))
    apool = ctx.enter_context(tc.tile_pool(name="att", bufs=2))
    pacc = ctx.enter_context(tc.tile_pool(name="pacc", bufs=2, space="PSUM"))
    ptrp = ctx.enter_context(tc.tile_pool(name="ptrp", bufs=2, space="PSUM"))
    pwork = ctx.enter_context(tc.tile_pool(name="pwork", bufs=4, space="PSUM"))

    # param loads ride the scalar queue so the first x load isn't delayed
    w1 = const.tile([P, 2 * (HID + 1)], FP32)
    nc.scalar.dma_start(w1[:], w1_d[:])
    b1 = const.tile([HID + 1, 1], FP32)
    nc.scalar.dma_start(b1[:], b1_d[:])
    w2b = const.tile([HID + 1, C], FP32)
    nc.scalar.dma_start(w2b[:], w2b_d[:])
    cmain = const.tile([P, 2 * P], FP16)
    nc.scalar.dma_start(cmain[:], cm_d[:])
    ccorn = const.tile([P, 4 * P], FP16)
    nc.scalar.dma_start(ccorn[:], cc_d[:])
    ones = const.tile([P, P], FP32)
    nc.scalar.dma_start(ones[:], ones_d[:])
    ident = const.tile([P, P], FP16)
    nc.scalar.dma_start(ident[:], id_d[:])
    redcol = const.tile([P, 1], FP16)
    nc.scalar.dma_start(redcol[:], rc_d[:])

    def spatial_fold1(xt, fa, op, half):
        """First c-fold (256 -> 128) for one DMA half of x (16 tiles)."""
        lo = F // 2 * half
        x4 = xt[:, lo:lo + F // 2].rearrange("p (t h c) -> p t h c",
                                             h=2, c=128)
        nc.vector.tensor_tensor(
            fa[:, 2048 * half:2048 * (half + 1)].rearrange(
                "p (t h c) -> p t h c", h=1, c=128),
            x4[:, :, 0:1, :], x4[:, :, 1:2, :], op=op)

    def spatial_rest(fa, fb, out, op, pool_tail=False):
        """Folds c: 128 -> 8 through fa/fb ping-pong, then a cheap tail.

        pool_tail moves the two small late folds to the idle Pool engine.
        """
        seq = [(fa, 4096), (fb, 2048), (fa, 1024), (fb, 512), (fa, 256)]
        for i in range(1, 5):
            sbuf, sw = seq[i - 1]
            dbuf, dw = seq[i]
            ch = sw // NT // 2
            s4 = sbuf[:, 0:sw].rearrange("p (t h c) -> p t h c", h=2, c=ch)
            eng = nc.gpsimd if (pool_tail and i >= 3) else nc.vector
            eng.tensor_tensor(
                dbuf[:, 0:dw].rearrange("p (t h c) -> p t h c", h=1, c=ch),
                s4[:, :, 0:1, :], s4[:, :, 1:2, :], op=op)
        with nc.allow_low_precision(reason="f16 spatial stats feed sigmoid"):
            nc.vector.tensor_reduce(
                out[:], fa[:, 0:256].rearrange("p (t c) -> p t c", c=8),
                axis=AX.X, op=op)

    def emit_stats(b):
        st = {}
        xt = xpool.tile([P, F], FP16, tag="x", name=f"x{b}")
        # batch 0's second half rides the idle gpsimd queue so both halves'
        # descriptor generation overlaps (fill time); later batches load
        # far enough ahead that the sync queue alone keeps up
        nc.sync.dma_start(xt[:, 0:F // 2], x_d[b, :, 0:F // 2])
        q2 = nc.gpsimd if b == 0 else nc.sync
        q2.dma_start(xt[:, F // 2:F], x_d[b, :, F // 2:F])
        st["xt"] = xt

        # channel sum over L (PE): 16 matmuls of [1, 512]; even tiles land
        # in cols 0:256, odd tiles in 256:512 (folded by DVE below)
        pcs2 = pacc.tile([1, 2 * C], FP32, tag="pcs")
        for j in range(NT // 2):
            nc.tensor.matmul(pcs2[:], redcol[:], xt[:, 2 * C * j:2 * C * (j + 1)],
                             start=(j == 0), stop=(j == NT // 2 - 1),
                             skip_group_check=True)

        # per-half first folds start as soon as each DMA half lands
        mb = mpool.tile([P, F // 2], FP16, tag="mb")
        fa = mpool.tile([P, 4096], FP16, tag="fa")
        fb = mpool.tile([P, 2048], FP16, tag="fb")
        ga = mpool.tile([P, 4096], FP16, tag="ga")
        gb = mpool.tile([P, 2048], FP16, tag="gb")
        for h in range(2):
            lo = F // 2 * h
            # chan-max: fold tiles {t, t+8} within this half
            nc.vector.tensor_max(mb[:, 2048 * h:2048 * (h + 1)],
                                 xt[:, lo:lo + 2048], xt[:, lo + 2048:lo + 4096])
            spatial_fold1(xt, fa, ALU.max, h)
            spatial_fold1(xt, ga, ALU.add, h)

        # chan-max tree: fold mb 4096 -> 256 (contiguous halves)
        w = F // 4
        while w >= C:
            nc.vector.tensor_max(mb[:, 0:w], mb[:, 0:w], mb[:, w:2 * w])
            w //= 2
        # cross-partition max via PE transposes + small DVE reduces
        pmaxT = ptrp.tile([P, 2 * P], FP16, tag="pmaxT")
        for h in range(2):
            nc.tensor.transpose(pmaxT[:, P * h:P * (h + 1)],
                                mb[:, P * h:P * (h + 1)], ident[:])
        stats_cm = spool.tile([P, 4], FP32, tag="stats_cm")
        for h in range(2):
            nc.vector.tensor_reduce(stats_cm[:, 2 * h + 1:2 * h + 2],
                                    pmaxT[:, P * h:P * (h + 1)],
                                    axis=AX.X, op=ALU.max)

        # spatial max + sum trees (DVE)
        max_s = spool.tile([P, NT], FP16, tag="max_s")
        spatial_rest(fa, fb, max_s, ALU.max)
        # avg fold: [1, 512] psum -> sbuf copy -> [1, 256] f16 (Pool add)
        sr512 = spool.tile([1, 2 * C], FP16, tag="sr512")
        nc.scalar.activation(sr512[:], pcs2[:], AF.Copy)
        avg_row = spool.tile([1, C], FP16, tag="avg_row")
        nc.gpsimd.tensor_add(avg_row[:], sr512[0:1, 0:C], sr512[0:1, C:2 * C])
        sum_s = spool.tile([P, NT], FP16, tag="sum_s")
        spatial_rest(ga, gb, sum_s, ALU.add)

        # avg transposes into channel-major + ACT copies
        pavgT = pwork.tile([P, 4], FP16, tag="pwork")
        for h in range(2):
            nc.tensor.transpose(pavgT[:, 2 * h:2 * h + 1],
                                avg_row[0:1, P * h:P * (h + 1)],
                                ident[0:1, 0:1])
        for h in range(2):
            nc.scalar.activation(stats_cm[:, 2 * h:2 * h + 1],
                                 pavgT[:, 2 * h:2 * h + 1], AF.Copy)

        # shared MLP (f32, tiny): row HID carries the 2*b2 constant
        ph = pwork.tile([HID + 1, 2], FP32, tag="pwork")
        nc.tensor.matmul(ph[:], w1[:, 0:HID + 1], stats_cm[:, 0:2],
                         start=True, stop=False, skip_group_check=True)
        nc.tensor.matmul(ph[:], w1[:, HID + 1:2 * (HID + 1)], stats_cm[:, 2:4],
                         start=False, stop=True, skip_group_check=True)
        hsb = spool.tile([HID + 1, 2], FP32, tag="hsb")
        nc.scalar.activation(hsb[:], ph[:], AF.Relu, bias=b1[:])
        h2 = spool.tile([HID + 1, 1], FP32, tag="h2")
        nc.vector.tensor_add(h2[:], hsb[:, 0:1], hsb[:, 1:2])
        h2r = spool.tile([HID + 1, P], FP32, tag="h2r")
        nc.vector.tensor_scalar_mul(h2r[:], ones[0:HID + 1, :], h2[:])
        po = pwork.tile([P, C], FP32, tag="pwork")
        nc.tensor.matmul(po[:], h2r[:], w2b[:], start=True, stop=True,
                         skip_group_check=True)
        att = apool.tile([P, C], FP16, tag="att")
        nc.scalar.activation(att[:], po[:], AF.Sigmoid)
        st["att"] = att

        # spatial conv over L: banded-Toeplitz matmuls (f16 in)
        pc = pwork.tile([P, NT], FP32, tag="pwork")
        nc.tensor.matmul(pc[:, :], cmain[:, 0:P], sum_s[:],
                         start=True, stop=False, skip_group_check=True)
        nc.tensor.matmul(pc[:, :], cmain[:, P:2 * P], max_s[:],
                         start=False, stop=False, skip_group_check=True)
        nc.tensor.matmul(pc[:, 1:NT], ccorn[:, 0:P], sum_s[:, 0:NT - 1],
                         start=False, stop=False, skip_group_check=True)
        nc.tensor.matmul(pc[:, 1:NT], ccorn[:, P:2 * P], max_s[:, 0:NT - 1],
                         start=False, stop=False, skip_group_check=True)
        nc.tensor.matmul(pc[:, 0:NT - 1], ccorn[0:3, 2 * P:3 * P],
                         sum_s[0:3, 1:NT],
                         start=False, stop=False, skip_group_check=True)
        nc.tensor.matmul(pc[:, 0:NT - 1], ccorn[0:3, 3 * P:4 * P],
                         max_s[0:3, 1:NT],
                         start=False, stop=True, skip_group_check=True)
        sig = apool.tile([P, NT], FP32, tag="sig")
        nc.scalar.activation(sig[:], pc[:], AF.Sigmoid)
        st["sig"] = sig
        st["b"] = b
        return st

    def emit_sq(st):
        # x^2: ACT for most columns (emitted after the previous batch's
        # final so the ACT stats chain isn't queued behind it), Pool tail
        sq = sqpool.tile([P, F], FP16, tag="sq")
        xt = st["xt"]
        nc.scalar.activation(sq[:, 0:SQ_ACT // 2], xt[:, 0:SQ_ACT // 2],
                             AF.Square)
        nc.scalar.activation(sq[:, SQ_ACT // 2:SQ_ACT],
                             xt[:, SQ_ACT // 2:SQ_ACT], AF.Square)
        if SQ_ACT < F:
            nc.gpsimd.tensor_tensor(sq[:, SQ_ACT:F], xt[:, SQ_ACT:F],
                                    xt[:, SQ_ACT:F], op=ALU.mult)
        st["sq"] = sq

    def emit_final(st, last=False):
        att, sig, sq = st["att"], st["sig"], st["sq"]
        # satt[:, 256t + c] = att[c] + sig[p, t], processed per half-batch
        # so each half's multiply and store start as soon as its satts land.
        # Within a half: DVE tensor_scalar for the first tiles, ACT
        # identity-with-bias for the rest; mul = DVE + small Pool tail.
        satt = stpool.tile([P, F], FP16, tag="satt")
        ot = opool.tile([P, F], FP16, tag="ot")
        half_t = NT // 2
        hf = F // 2
        sdve = half_t // 2 if last else SATT_DVE // 2
        pool_cols = 512
        for hh in range(2):
            t0 = half_t * hh
            for t in range(t0, t0 + sdve):
                nc.vector.tensor_scalar_add(satt[:, C * t:C * (t + 1)],
                                            att[:], sig[:, t:t + 1])
            for t in range(t0 + sdve, t0 + half_t):
                nc.scalar.activation(satt[:, C * t:C * (t + 1)], att[:],
                                     AF.Identity, bias=sig[:, t:t + 1])
            lo = hf * hh
            nc.vector.tensor_mul(ot[:, lo:lo + hf - pool_cols],
                                 satt[:, lo:lo + hf - pool_cols],
                                 sq[:, lo:lo + hf - pool_cols])
            nc.gpsimd.tensor_tensor(ot[:, lo + hf - pool_cols:lo + hf],
                                    satt[:, lo + hf - pool_cols:lo + hf],
                                    sq[:, lo + hf - pool_cols:lo + hf],
                                    op=ALU.mult)
            nc.gpsimd.dma_start(out_d[st["b"], :, lo:lo + hf],
                                ot[:, lo:lo + hf])

    # software-pipelined emission: final(b-1) lands after stats(b) so no
    # engine stalls head-of-line on the cross-engine satt join; sq(b) goes
    # last so the ACT stats chain isn't queued behind it
    prev = None
    for b in [b for _ in range(reps) for b in range(NB)]:
        cur = emit_stats(b)
        if prev is not None:
            emit_final(prev)
        emit_sq(cur)
        prev = cur
    emit_final(prev, last=True)


def _build_nc(reps=1):
    nc = bacc.Bacc("TRN2", target_bir_lowering=False, debug=False,
                   enable_asserts=False, num_devices=N_CORES)
    x_d = nc.dram_tensor("xb", [NB, P, F], FP16, kind="ExternalInput").ap()
    w1_d = nc.dram_tensor("w1sb", [P, 2 * (HID + 1)], FP32, kind="ExternalInput").ap()
    b1_d = nc.dram_tensor("b1col", [HID + 1, 1], FP32, kind="ExternalInput").ap()
    w2b_d = nc.dram_tensor("w2b", [HID + 1, C], FP32, kind="ExternalInput").ap()
    cm_d = nc.dram_tensor("convmain", [P, 2 * P], FP16, kind="ExternalInput").ap()
    cc_d = nc.dram_tensor("convcorner", [P, 4 * P], FP16, kind="ExternalInput").ap()
    ones_d = nc.dram_tensor("ones", [P, P], FP32, kind="ExternalInput").ap()
    id_d = nc.dram_tensor("ident", [P, P], FP16, kind="ExternalInput").ap()
    rc_d = nc.dram_tensor("redcol", [P, 1], FP16, kind="ExternalInput").ap()
    out_d = nc.dram_tensor("out", [NB, P, F], FP16, kind="ExternalOutput").ap()

    with tile.TileContext(nc) as tc:
        with ExitStack() as ctx:
            _build_body(ctx, tc, out_d, x_d, w1_d, b1_d, w2b_d, cm_d, cc_d,
                        ones_d, id_d, rc_d, reps=reps)
    nc.compile()
    return nc


def get_nc(reps=1):
    key = f"nc{reps}"
    if key not in _CACHE:
        _CACHE[key] = _build_nc(reps=reps)
    return _CACHE[key]


def _prep_inputs(W1, b1, W2, b2, conv_w):
    """Host-side parameter preprocessing (shared across cores)."""
    W1 = np.asarray(W1, np.float32)
    W2 = np.asarray(W2, np.float32)
    b1 = np.asarray(b1, np.float32)
    b2 = np.asarray(b2, np.float32)
    conv_w = np.asarray(conv_w, np.float32)

    HB = HID + 1
    w1sb = np.zeros((P, 2 * HB), np.float32)
    for h in range(2):
        w1sb[:, HB * h:HB * h + HID] = W1[P * h:P * (h + 1), :]
    w2b = np.concatenate([W2, b2[None, :]], axis=0).astype(np.float32)
    b1col = np.concatenate([b1, [1.0]]).astype(np.float32).reshape(HB, 1)

    # Banded Toeplitz over two adjacent 128-blocks; avg band folds in the
    # 1/C spatial-mean scale (device computes raw channel sums).
    wa = conv_w[:, 0, 0] / C
    wm = conv_w[:, 1, 0]
    Wb_a = np.zeros((2 * P, 2 * P), np.float32)
    Wb_m = np.zeros((2 * P, 2 * P), np.float32)
    for i in range(2 * P):
        for k in range(7):
            j = i + k - 3
            if 0 <= j < 2 * P:
                Wb_a[i, j] = wa[k]
                Wb_m[i, j] = wm[k]
    cmain = np.concatenate([Wb_a[0:P, 0:P].T, Wb_m[0:P, 0:P].T], axis=1)
    # Corner lhsTs in one [128, 512] tensor. The prev-block ("lo") bands use
    # full K=128 (only rows 125-127 nonzero) so the rhs stays at base
    # partition 0 (PE requires base partition in {0, 32, 64}); the
    # next-block ("hi") bands are K=3 at rows 0-2.
    corn = np.zeros((P, 4 * P), np.float32)
    corn[:, 0:P] = Wb_a[P:2 * P, 0:P].T            # prev-block avg
    corn[:, P:2 * P] = Wb_m[P:2 * P, 0:P].T        # prev-block max
    corn[0:3, 2 * P:3 * P] = Wb_a[0:P, P:2 * P].T[0:3, :]   # next-block avg
    corn[0:3, 3 * P:4 * P] = Wb_m[0:P, P:2 * P].T[0:3, :]   # next-block max
    return {
        "w1sb": w1sb,
        "b1col": np.ascontiguousarray(b1col),
        "w2b": w2b,
        "convmain": np.ascontiguousarray(cmain).astype(np.float16),
        "convcorner": np.ascontiguousarray(corn).astype(np.float16),
        "ones": np.ones((P, P), np.float32),
        "ident": np.eye(P, dtype=np.float16),
        "redcol": np.full((P, 1), 1.0 / L, np.float16),
    }


def kernel(x, W1, b1, W2, b2, conv_w):
    nc = get_nc()
    x = np.asarray(x, np.float32)
    params = _prep_inputs(W1, b1, W2, b2, conv_w)
    # Stage x as f16 in the SBUF tile layout: [NB, 128, NT*C] with
    # col = 256 * (l // 128) + c, partition = l % 128.
    xt = x.reshape(B_FULL, NT, P, C).transpose(0, 2, 1, 3).reshape(
        B_FULL, P, F).astype(np.float16)
    in_maps = []
    for c in range(N_CORES):
        m = dict(params)
        m["xb"] = np.ascontiguousarray(xt[NB * c:NB * (c + 1)])
        in_maps.append(m)
    _CACHE["last_in_maps"] = in_maps
    res = run_bass_kernel_spmd(nc, in_maps, list(range(N_CORES)))
    _CACHE["last_results"] = res
    out = np.concatenate([res.results[c]["out"] for c in range(N_CORES)],
                         axis=0)
    # [B, 128, NT*C] f16 -> [B, L, C] f32
    return out.reshape(B_FULL, P, NT, C).transpose(0, 2, 1, 3).reshape(
        B_FULL, L, C).astype(np.float32)


def _pjrt_exec(nc, in_maps, n_warm=2, n_time=8):
    """Build a sharded jit for nc, run it, return (best_wall_s, result)."""
    import time
    import jax
    import concourse.mybir as mybir_
    from concourse.bass2jax import (_bass_exec_p, install_neuronx_cc_hook,
                                    partition_id_tensor)
    from jax.experimental.shard_map import shard_map
    from jax.sharding import Mesh, PartitionSpec

    install_neuronx_cc_hook()
    partition_name = (nc.partition_id_tensor.name
                      if nc.partition_id_tensor else None)
    in_names, out_names, out_avals = [], [], []
    for alloc in nc.m.functions[0].allocations:
        if not isinstance(alloc, mybir_.MemoryLocationSet):
            continue
        name = alloc.memorylocations[0].name
        if alloc.kind == "ExternalInput":
            if name != partition_name:
                in_names.append(name)
        elif alloc.kind == "ExternalOutput":
            out_names.append(name)
            out_avals.append(jax.core.ShapedArray(
                tuple(alloc.tensor_shape), mybir_.dt.np(alloc.dtype)))
    n_params = len(in_names)
    all_in_names = list(in_names) + list(out_names)
    if partition_name is not None:
        all_in_names.append(partition_name)

    def _body(*args):
        operands = list(args)
        if partition_name is not None:
            operands.append(partition_id_tensor())
        return tuple(_bass_exec_p.bind(
            *operands,
            out_avals=tuple(out_avals),
            in_names=tuple(all_in_names),
            out_names=tuple(out_names),
            lowering_input_output_aliases=(),
            sim_require_finite=True,
            sim_require_nnan=True,
            nc=nc,
        ))

    devices = jax.devices()[:N_CORES]
    mesh = Mesh(np.asarray(devices), ("core",))
    nin = n_params + len(out_names)
    sharding = jax.sharding.NamedSharding(mesh, PartitionSpec("core"))
    fn = jax.jit(shard_map(
        _body, mesh=mesh,
        in_specs=(PartitionSpec("core"),) * nin,
        out_specs=(PartitionSpec("core"),) * len(out_names),
        check_rep=False))
    dev_args = [
        jax.device_put(np.concatenate(
            [np.asarray(in_maps[c][nm]) for c in range(N_CORES)], axis=0),
            sharding)
        for nm in in_names
    ]
    for av in out_avals:
        z = np.zeros((N_CORES * av.shape[0], *av.shape[1:]), av.dtype)
        dev_args.append(jax.device_put(z, sharding))

    for _ in range(n_warm):
        out = fn(*dev_args)
        jax.block_until_ready(out)
    best = float("inf")
    for _ in range(n_time):
        t0 = time.perf_counter()
        out = fn(*dev_args)
        jax.block_until_ready(out)
        best = min(best, time.perf_counter() - t0)
    result = np.asarray(out[0])
    return best, result


def bench_repeat(reps=8, n_time=10, in_maps=None):
    """Isolate device exec time: time a module doing the work `reps` times
    in-kernel vs once; slope = steady-state HW time per execution."""
    if in_maps is None:
        in_maps = _CACHE["last_in_maps"]
    t1, _ = _pjrt_exec(get_nc(1), in_maps, n_time=n_time)
    tr, result = _pjrt_exec(get_nc(reps), in_maps, n_time=n_time)
    per_exec_ns = (tr - t1) / (reps - 1) * 1e9
    return per_exec_ns, result, t1 * 1e9, tr * 1e9
